# revision 41
# baseline (speedup 1.0000x reference)
"""Causal self-attention on 8 trn2 NeuronCores.

Sharding: core = 2*b + g  (b in 0..3 batches, g in 0..1 head-groups of 8
heads). Each core computes, for its batch b and its 8 heads:
  qkv^T = W_slice^T @ x_b^T   (x^T provided by host; feature-major)
  per-head causal softmax attention (scores^T layout, ones-augmented V
  accumulates the softmax denominator in the same matmul)
  partial out^T = wp_slice^T @ y^T  -> [1024, 2048] bf16
Host gathers: out[b] = (partial[2b] + partial[2b+1]).T + b_proj.

v2: host-side x transpose, s0/s1 score-matmul pairing (PE row-group
concurrency), causal trimming, triangle-mask on DVE, fast reciprocal,
bf16 output, QKV emission interleaved into attention for engine overlap.
"""

import numpy as np
import ml_dtypes

B, T, E, H = 4, 2048, 1024, 16
HD = E // H  # 64

_CACHE = {}


def _build():
    from contextlib import ExitStack

    import concourse.bass as bass
    import concourse.mybir as mybir
    import concourse.tile as tile
    from concourse import bacc

    F32 = mybir.dt.float32
    BF16 = mybir.dt.bfloat16
    AF = mybir.ActivationFunctionType
    MUL = mybir.AluOpType.mult

    nc = bacc.Bacc("TRN2", target_bir_lowering=False)
    xinT = nc.dram_tensor("xinT", [E, T], BF16, kind="ExternalInput")
    wqkv = nc.dram_tensor("wqkv", [12, 128, 8, 128], BF16, kind="ExternalInput")
    bqkv = nc.dram_tensor("bqkv", [128, 12], F32, kind="ExternalInput")
    wp = nc.dram_tensor("wp", [128, 4, 1024], BF16, kind="ExternalInput")
    outT = nc.dram_tensor("outT", [E, T], BF16, kind="ExternalOutput")

    with tile.TileContext(nc) as tc, ExitStack() as ctx:
        const = ctx.enter_context(tc.tile_pool(name="const", bufs=1))
        # stacked 64x64 identities at partition 0 and 64 (for v-transpose)
        id2f = const.tile([128, 64], F32, tag="id2f")
        nc.gpsimd.memset(id2f[:], 0.0)
        for off in (0, 64):
            nc.gpsimd.affine_select(
                out=id2f[:],
                in_=id2f[:],
                compare_op=mybir.AluOpType.not_equal,
                fill=1.0,
                base=-off,
                pattern=[[-1, 64]],
                channel_multiplier=1,
            )
        id2 = const.tile([128, 64], BF16, tag="id2")
        nc.vector.tensor_copy(id2[:], id2f[:])
        # causal triangle mask [128,128]: tri[k, q] = 1 if q >= k else 0
        trif = const.tile([128, 128], F32, tag="trif")
        nc.gpsimd.memset(trif[:], 1.0)
        nc.gpsimd.affine_select(
            out=trif[:],
            in_=trif[:],
            compare_op=mybir.AluOpType.is_ge,
            fill=0.0,
            base=0,
            pattern=[[1, 128]],
            channel_multiplier=-1,
        )
        tri = const.tile([128, 128], BF16, tag="tri")
        nc.vector.tensor_copy(tri[:], trif[:])
        biasT = const.tile([128, 12], F32, tag="biasT")
        nc.sync.dma_start(biasT[:], bqkv[:])

        # persistent SBUF tensors (wqm DMAs for p=0 issued before the bulk
        # xT load so the first matmuls are not starved)
        wqf_pool = ctx.enter_context(tc.tile_pool(name="wqf", bufs=3))
        wqms = {}
        for m in range(3):
            wqm = wqf_pool.tile([128, 8, 128], BF16, tag="wqm", name=f"wqm{m}")
            nc.sync.dma_start(wqm[:], wqkv[m])
            wqms[m] = wqm

        xT_pool = ctx.enter_context(tc.tile_pool(name="xT", bufs=1))
        xT = xT_pool.tile([128, 8, T], BF16, tag="xT")
        for k in range(8):
            nc.sync.dma_start(xT[:, k, :], xinT[k * 128 : (k + 1) * 128, :])

        wpp_pool = ctx.enter_context(tc.tile_pool(name="wpp", bufs=1))
        wps = wpp_pool.tile([128, 4, 1024], BF16, tag="wps")
        nc.sync.dma_start(wps[:], wp[:])

        qkvT_pool = ctx.enter_context(tc.tile_pool(name="qkvT", bufs=1))
        qkvTp = [
            qkvT_pool.tile([128, 3, T], BF16, tag=f"qkvT{p}", name=f"qkvT{p}")
            for p in range(4)
        ]
        yT_pool = ctx.enter_context(tc.tile_pool(name="yT", bufs=1))
        yTp = [
            yT_pool.tile([128, T], BF16, tag=f"yT{p}", name=f"yT{p}")
            for p in range(4)
        ]

        with (
            tc.tile_pool(name="wq", bufs=3) as wq_pool,
            tc.tile_pool(name="vaug", bufs=2) as vaug_pool,
            tc.tile_pool(name="Pp", bufs=26) as P_pool,
            tc.tile_pool(name="smallB", bufs=2) as smallB,
            tc.tile_pool(name="pssp", bufs=2, space="PSUM") as pssp,
            tc.tile_pool(name="psy", bufs=2, space="PSUM") as psy,
        ):
            # ---- emission helpers ------------------------------------
            def emit_qkv_chunk(p, r, half, wqm):
                """one [128,1024] output chunk of the qkv projection:
                m = 3p+r, T-half `half`."""
                m = 3 * p + r
                pq = pssp.tile([128, 1024], F32, tag="sp")
                t0 = half * 1024
                for k in range(8):
                    for j in range(2):
                        nc.tensor.matmul(
                            pq[:, j * 512 : (j + 1) * 512],
                            wqm[:, k, :],
                            xT[:, k, t0 + j * 512 : t0 + (j + 1) * 512],
                            start=(k == 0),
                            stop=(k == 7),
                        )
                nc.vector.tensor_scalar_add(
                    qkvTp[p][:, r, t0 : t0 + 1024], pq[:], biasT[:, m : m + 1]
                )

            def emit_vtrans(p, vaug, kbs=range(16)):
                """v^T -> vaug [128k, s, 16*65] with ones in col 64.
                s inner so s0/s1 run on concurrent PE row-groups."""
                for kb in kbs:
                    pv = pssp.tile([128, 1024], F32, tag="sp")
                    for s in range(2):
                        # s0/s1 outputs land in different PSUM banks
                        nc.tensor.matmul(
                            pv[:, 512 * s : 512 * s + 64],
                            qkvTp[p][
                                64 * s : 64 * s + 64, 2, kb * 128 : (kb + 1) * 128
                            ],
                            id2[64 * s : 64 * s + 64, :],
                            start=True,
                            stop=True,
                            tile_position=(64 * s, 0),
                        )
                    for s in range(2):
                        nc.vector.tensor_copy(
                            vaug[:, s, kb * 65 : kb * 65 + 64],
                            pv[:, 512 * s : 512 * s + 64],
                        )

            # A(0) upfront: qkv for p=0 (wqm tiles were DMA'd at the top).
            # m=0,1 are emitted k-outer so the PE tracks the incremental xT
            # DMA arrivals instead of stalling on the full 4MB load.
            pq01 = [
                pssp.tile([128, 1024], F32, tag="sp", name=f"pq01_{i}")
                for i in range(2)
            ] + [
                psy.tile([128, 1024], F32, tag="y", name=f"pq01_{2 + i}")
                for i in range(2)
            ]
            for k in range(8):
                for mi in range(2):
                    for half in range(2):
                        for j in range(2):
                            c = half * 1024 + j * 512
                            nc.tensor.matmul(
                                pq01[2 * mi + half][:, j * 512 : (j + 1) * 512],
                                wqms[mi][:, k, :],
                                xT[:, k, c : c + 512],
                                start=(k == 0),
                                stop=(k == 7),
                            )
            for mi in range(2):
                for half in range(2):
                    nc.vector.tensor_scalar_add(
                        qkvTp[0][:, mi, half * 1024 : half * 1024 + 1024],
                        pq01[2 * mi + half][:],
                        biasT[:, mi : mi + 1],
                    )
            del wqms[0], wqms[1]
            for half in range(2):
                emit_qkv_chunk(0, 2, half, wqms[2])
            del wqms[2]
            vaugs = {}

            # filler-work generator: qkv chunks + vtrans for p+1 emitted
            # lazily inside B(p)'s kb loop
            def make_filler(pnext):
                if pnext > 3:
                    def _none():
                        return
                    return _none
                state = {"stage": 0, "r": 0, "half": 0, "vkb": 0}

                def step():
                    st = state["stage"]
                    if st == 0:
                        # DMA all three weight slices up front
                        for r in range(3):
                            m = 3 * pnext + r
                            wqm = wq_pool.tile(
                                [128, 8, 128], BF16, tag="wqm", name=f"wqm{m}"
                            )
                            nc.sync.dma_start(wqm[:], wqkv[m])
                            wqms[m] = wqm
                        state["stage"] = 1
                    elif st == 1:
                        r, half = state["r"], state["half"]
                        emit_qkv_chunk(pnext, r, half, wqms[3 * pnext + r])
                        if half == 1:
                            del wqms[3 * pnext + r]
                            state["r"], state["half"] = r + 1, 0
                            if r + 1 == 3:
                                state["stage"] = 2
                        else:
                            state["half"] = 1
                    # stage 2: exhausted

                return step

            # ------------- attention: chain-pipelined -------------
            # Each (p, qc, s) is a chain. A chain's scores+exp stream is
            # ACT-paced; its PV matmuls (pure PE work, all deps satisfied
            # once the exps ran) are emitted interleaved into the NEXT
            # chain's scores stream, keeping both engines dense.
            def geometry(qc):
                out = []
                for kb in range((qc + 1) * 8):
                    diag = kb >= qc * 8
                    q_lo = qc * 1024 if not diag else (kb * 128 // 512) * 512
                    w = (qc + 1) * 1024 - q_lo
                    dd = kb * 128 - q_lo if diag else 0  # in {0,128,256,384}
                    out.append((kb, q_lo, w, dd, diag))
                return out

            def emit_pv_step(ch, job):
                kb, q_lo, w, dd, diag = job
                for j in range(w // 512):
                    col = q_lo - ch["qc"] * 1024 + j * 512
                    ci = col // 512
                    nc.tensor.matmul(
                        ch["ymm"][0:65, col : col + 512],
                        ch["vaug"][:, ch["s"], kb * 65 : kb * 65 + 65],
                        ch["Pts"][kb][:, j * 512 : (j + 1) * 512],
                        start=(kb == 0),
                        stop=(kb == ch["klast"][ci]),
                    )

            def emit_normalize(ch):
                p_, qc_, s_ = ch["p"], ch["qc"], ch["s"]
                sfx = f"{p_}{qc_}{s_}"
                den = smallB.tile([1, 1024], F32, tag="den", name=f"den{sfx}")
                nc.vector.tensor_copy(den[0:1, :], ch["ymm"][64:65, :])
                rec = smallB.tile([1, 1024], F32, tag="rec", name=f"rec{sfx}")
                nc.vector.reciprocal_approx_fast(rec[0:1, :], den[0:1, :])
                bcs = smallB.tile([64, 1024], F32, tag="bcs", name=f"bcs{sfx}")
                nc.gpsimd.partition_broadcast(bcs[:], rec[0:1, :])
                nc.vector.tensor_tensor(
                    out=yTp[p_][
                        64 * s_ : 64 * s_ + 64, qc_ * 1024 : (qc_ + 1) * 1024
                    ],
                    in0=ch["ymm"][0:64, :],
                    in1=bcs[:],
                    op=MUL,
                )

            prev = None
            filler = None
            gstep = 0
            vt = 0
            for p in range(4):
                for qc in range(2):
                    for s in range(2):
                        if (qc, s) == (0, 0):
                            filler = make_filler(p + 1)
                            vaug = vaug_pool.tile(
                                [128, 2, 16 * 65], BF16, tag="vaug", name=f"vg{p}"
                            )
                            nc.gpsimd.memset(vaug[:], 1.0)
                            vaugs[p] = vaug
                            vt = 0
                        steps = geometry(qc)
                        klast = [
                            min(len(steps) - 1, (qc * 2 + ci + 1) * 4 - 1)
                            for ci in range(2)
                        ]
                        qT = qkvTp[p][64 * s : 64 * s + 64, 0, :]
                        kT = qkvTp[p][64 * s : 64 * s + 64, 1, :]
                        cur = {
                            "p": p,
                            "qc": qc,
                            "s": s,
                            "klast": klast,
                            "vaug": vaugs[p],
                            "Pts": {},
                            "steps": steps,
                        }
                        if prev is not None:
                            prev["ymm"] = psy.tile(
                                [128, 1024],
                                F32,
                                tag="y",
                                name=f"y{prev['p']}_{prev['qc']}_{prev['s']}",
                            )
                        npv = len(prev["steps"]) if prev is not None else 0
                        done_pv = 0
                        for i, job in enumerate(steps):
                            kb, q_lo, w, dd, diag = job
                            spt = pssp.tile(
                                [128, 1024], F32, tag="sp", name=f"sp{p}_{qc}_{s}_{kb}"
                            )
                            for j in range(w // 512):
                                c0 = j * 512 + (dd if j == 0 else 0)
                                c1 = (j + 1) * 512
                                nc.tensor.matmul(
                                    spt[:, c0:c1],
                                    kT[:, kb * 128 : (kb + 1) * 128],
                                    qT[:, q_lo + c0 : q_lo + c1],
                                    start=True,
                                    stop=True,
                                    tile_position=(64 * s, 0),
                                )
                            Pts = P_pool.tile(
                                [128, 1024], BF16, tag="P", name=f"P{p}_{qc}_{s}_{kb}"
                            )
                            cur["Pts"][kb] = Pts
                            nc.scalar.activation(
                                Pts[:, dd:w], spt[:, dd:w], AF.Exp, scale=0.125
                            )
                            if diag:
                                if dd > 0:
                                    nc.vector.memset(Pts[:, 0:dd], 0.0)
                                nc.vector.tensor_mul(
                                    Pts[:, dd : dd + 128],
                                    Pts[:, dd : dd + 128],
                                    tri[:],
                                )
                            # interleave the previous chain's PV work, paced
                            if prev is not None:
                                tgt = (i + 1) * npv // len(steps)
                                while done_pv < tgt:
                                    emit_pv_step(prev, prev["steps"][done_pv])
                                    done_pv += 1
                            # v-transpose for this p: 2 blocks per (qc0,s0) step
                            if qc == 0 and s == 0 and vt < 16:
                                emit_vtrans(p, vaugs[p], range(vt, vt + 2))
                                vt += 2
                            gstep += 1
                            if gstep % 3 == 0:
                                filler()
                        if prev is not None:
                            while done_pv < npv:
                                emit_pv_step(prev, prev["steps"][done_pv])
                                done_pv += 1
                            emit_normalize(prev)
                        prev = cur
                for _ in range(8):
                    filler()
            # final chain's PV + normalize
            prev["ymm"] = psy.tile([128, 1024], F32, tag="y", name="y_last")
            for job in prev["steps"]:
                emit_pv_step(prev, job)
            emit_normalize(prev)

        # ---------------- output projection ----------------
        with (
            tc.tile_pool(name="ob", bufs=2) as ob_pool,
            tc.tile_pool(name="psC", bufs=8, space="PSUM") as psC,
        ):
            for m in range(8):
                pn = [
                    psC.tile([128, 512], F32, tag="pc", name=f"pc{m}_{n}")
                    for n in range(4)
                ]
                for k in range(4):
                    for n in range(4):
                        nc.tensor.matmul(
                            pn[n][:],
                            wps[:, k, m * 128 : (m + 1) * 128],
                            yTp[k][:, n * 512 : (n + 1) * 512],
                            start=(k == 0),
                            stop=(k == 3),
                        )
                ob = ob_pool.tile([128, T], BF16, tag="ob")
                for n in range(4):
                    nc.vector.tensor_copy(ob[:, n * 512 : (n + 1) * 512], pn[n][:])
                nc.sync.dma_start(outT[m * 128 : (m + 1) * 128, :], ob[:])

    nc.compile()
    return nc


def _get_nc():
    if "nc" not in _CACHE:
        _CACHE["nc"] = _build()
    return _CACHE["nc"]


def _prep_core_inputs(x, w_attn, b_attn, w_proj, b, g):
    cols = []
    for p in range(4):
        off = 512 * g + 128 * p
        cols += [
            w_attn[:, off : off + 128],
            w_attn[:, E + off : E + off + 128],
            w_attn[:, 2 * E + off : 2 * E + off + 128],
        ]
    wq = np.concatenate(cols, axis=1)  # [1024, 1536]
    # -> [12, 128, 8, 128]: m-major so each per-m DMA slice is contiguous
    wq = np.ascontiguousarray(
        wq.reshape(8, 128, 12, 128).transpose(2, 1, 0, 3), dtype=np.float32
    )
    bcols = []
    for p in range(4):
        off = 512 * g + 128 * p
        bcols += [
            b_attn[off : off + 128],
            b_attn[E + off : E + off + 128],
            b_attn[2 * E + off : 2 * E + off + 128],
        ]
    bq = np.stack(bcols, axis=1).astype(np.float32)  # [128, 12]
    wpr = np.concatenate(
        [w_proj[512 * g + 128 * p : 512 * g + 128 * p + 128, :] for p in range(4)],
        axis=0,
    )  # [512, 1024]
    wpr = np.ascontiguousarray(
        wpr.reshape(4, 128, 1024).transpose(1, 0, 2), dtype=np.float32
    )
    return {
        "xinT": np.ascontiguousarray(x[b].T).astype(ml_dtypes.bfloat16),
        "wqkv": wq.astype(ml_dtypes.bfloat16),
        "bqkv": np.ascontiguousarray(bq),
        "wp": wpr.astype(ml_dtypes.bfloat16),
    }


def kernel(x, w_attn, b_attn, w_proj, b_proj, _trace=False):
    from concourse.bass_utils import run_bass_kernel_spmd

    x = np.asarray(x, dtype=np.float32)
    w_attn = np.asarray(w_attn, dtype=np.float32)
    b_attn = np.asarray(b_attn, dtype=np.float32)
    w_proj = np.asarray(w_proj, dtype=np.float32)
    b_proj = np.asarray(b_proj, dtype=np.float32)

    nc = _get_nc()
    in_maps = [
        _prep_core_inputs(x, w_attn, b_attn, w_proj, core // 2, core % 2)
        for core in range(8)
    ]
    res = run_bass_kernel_spmd(
        nc, in_maps, core_ids=list(range(8)), trace=_trace
    )
    _CACHE["last_results"] = res
    out = np.empty((B, T, E), dtype=np.float32)
    for b in range(B):
        acc = res.results[2 * b]["outT"].astype(np.float32) + res.results[
            2 * b + 1
        ]["outT"].astype(np.float32)
        out[b] = acc.T + b_proj[None, :]
    return out


# revision 45
# speedup vs baseline: 1.0771x; 1.0771x over previous
"""Causal self-attention on 8 trn2 NeuronCores.

Sharding: core = 2*b + g  (b in 0..3 batches, g in 0..1 head-groups of 8
heads). Each core computes, for its batch b and its 8 heads:
  qkv^T = W_slice^T @ x_b^T   (x^T provided by host; feature-major)
  per-head causal softmax attention (scores^T layout, ones-augmented V
  accumulates the softmax denominator in the same matmul)
  partial out^T = wp_slice^T @ y^T  -> [1024, 2048] bf16
Host gathers: out[b] = (partial[2b] + partial[2b+1]).T + b_proj.

v2: host-side x transpose, s0/s1 score-matmul pairing (PE row-group
concurrency), causal trimming, triangle-mask on DVE, fast reciprocal,
bf16 output, QKV emission interleaved into attention for engine overlap.
"""

import numpy as np
import ml_dtypes

B, T, E, H = 4, 2048, 1024, 16
HD = E // H  # 64

_CACHE = {}


def _build():
    from contextlib import ExitStack

    import concourse.bass as bass
    import concourse.mybir as mybir
    import concourse.tile as tile
    from concourse import bacc

    F32 = mybir.dt.float32
    BF16 = mybir.dt.bfloat16
    AF = mybir.ActivationFunctionType
    MUL = mybir.AluOpType.mult

    nc = bacc.Bacc("TRN2", target_bir_lowering=False)
    xinT = nc.dram_tensor("xinT", [E, T], BF16, kind="ExternalInput")
    wqkv = nc.dram_tensor("wqkv", [12, 128, 8, 128], BF16, kind="ExternalInput")
    bqkv = nc.dram_tensor("bqkv", [128, 12], F32, kind="ExternalInput")
    wp = nc.dram_tensor("wp", [128, 4, 1024], BF16, kind="ExternalInput")
    outT = nc.dram_tensor("outT", [E, T], BF16, kind="ExternalOutput")

    with tile.TileContext(nc) as tc, ExitStack() as ctx:
        const = ctx.enter_context(tc.tile_pool(name="const", bufs=1))
        # stacked 64x64 identities at partition 0 and 64 (for v-transpose)
        id2f = const.tile([128, 64], F32, tag="id2f")
        nc.gpsimd.memset(id2f[:], 0.0)
        for off in (0, 64):
            nc.gpsimd.affine_select(
                out=id2f[:],
                in_=id2f[:],
                compare_op=mybir.AluOpType.not_equal,
                fill=1.0,
                base=-off,
                pattern=[[-1, 64]],
                channel_multiplier=1,
            )
        id2 = const.tile([128, 64], BF16, tag="id2")
        nc.vector.tensor_copy(id2[:], id2f[:])
        # causal triangle mask [128,128]: tri[k, q] = 1 if q >= k else 0
        trif = const.tile([128, 128], F32, tag="trif")
        nc.gpsimd.memset(trif[:], 1.0)
        nc.gpsimd.affine_select(
            out=trif[:],
            in_=trif[:],
            compare_op=mybir.AluOpType.is_ge,
            fill=0.0,
            base=0,
            pattern=[[1, 128]],
            channel_multiplier=-1,
        )
        tri = const.tile([128, 128], BF16, tag="tri")
        nc.vector.tensor_copy(tri[:], trif[:])
        biasT = const.tile([128, 12], F32, tag="biasT")
        nc.sync.dma_start(biasT[:], bqkv[:])

        # persistent SBUF tensors (wqm DMAs for p=0 issued before the bulk
        # xT load so the first matmuls are not starved)
        wqf_pool = ctx.enter_context(tc.tile_pool(name="wqf", bufs=3))
        wqms = {}
        for m in range(3):
            wqm = wqf_pool.tile([128, 8, 128], BF16, tag="wqm", name=f"wqm{m}")
            nc.sync.dma_start(wqm[:], wqkv[m])
            wqms[m] = wqm

        xT_pool = ctx.enter_context(tc.tile_pool(name="xT", bufs=1))
        xT = xT_pool.tile([128, 8, T], BF16, tag="xT")
        for k in range(8):
            nc.sync.dma_start(xT[:, k, :], xinT[k * 128 : (k + 1) * 128, :])

        wpp_pool = ctx.enter_context(tc.tile_pool(name="wpp", bufs=1))
        wps = wpp_pool.tile([128, 4, 1024], BF16, tag="wps")
        nc.sync.dma_start(wps[:], wp[:])

        qkvT_pool = ctx.enter_context(tc.tile_pool(name="qkvT", bufs=1))
        qkvTp = [
            qkvT_pool.tile([128, 3, T], BF16, tag=f"qkvT{p}", name=f"qkvT{p}")
            for p in range(4)
        ]
        yT_pool = ctx.enter_context(tc.tile_pool(name="yT", bufs=1))
        yTp = [
            yT_pool.tile([128, T], BF16, tag=f"yT{p}", name=f"yT{p}")
            for p in range(4)
        ]

        with (
            tc.tile_pool(name="wq", bufs=3) as wq_pool,
            tc.tile_pool(name="vaug", bufs=2) as vaug_pool,
            tc.tile_pool(name="Pp", bufs=10) as P_pool,
            tc.tile_pool(name="smallB", bufs=4) as smallB,
            tc.tile_pool(name="pssp", bufs=2, space="PSUM") as pssp,
            tc.tile_pool(name="psy", bufs=2, space="PSUM") as psy,
        ):
            # ---- emission helpers ------------------------------------
            def emit_qkv_chunk(p, r, half, wqm):
                """one [128,1024] output chunk of the qkv projection:
                m = 3p+r, T-half `half`."""
                m = 3 * p + r
                pq = pssp.tile([128, 1024], F32, tag="sp")
                t0 = half * 1024
                for k in range(8):
                    for j in range(2):
                        nc.tensor.matmul(
                            pq[:, j * 512 : (j + 1) * 512],
                            wqm[:, k, :],
                            xT[:, k, t0 + j * 512 : t0 + (j + 1) * 512],
                            start=(k == 0),
                            stop=(k == 7),
                        )
                nc.vector.tensor_scalar_add(
                    qkvTp[p][:, r, t0 : t0 + 1024], pq[:], biasT[:, m : m + 1]
                )

            def emit_vtrans(p, vaug, kbs=range(16)):
                """v^T -> vaug [128k, s, 16*65] with ones in col 64.
                s inner so s0/s1 run on concurrent PE row-groups."""
                for kb in kbs:
                    pv = pssp.tile([128, 1024], F32, tag="sp")
                    for s in range(2):
                        # s0/s1 outputs land in different PSUM banks
                        nc.tensor.matmul(
                            pv[:, 512 * s : 512 * s + 64],
                            qkvTp[p][
                                64 * s : 64 * s + 64, 2, kb * 128 : (kb + 1) * 128
                            ],
                            id2[64 * s : 64 * s + 64, :],
                            start=True,
                            stop=True,
                            tile_position=(64 * s, 0),
                        )
                    for s in range(2):
                        nc.vector.tensor_copy(
                            vaug[:, s, kb * 65 : kb * 65 + 64],
                            pv[:, 512 * s : 512 * s + 64],
                        )

            # A(0) upfront: qkv for p=0 (wqm tiles were DMA'd at the top).
            # m=0,1 are emitted k-outer so the PE tracks the incremental xT
            # DMA arrivals instead of stalling on the full 4MB load.
            pq01 = [
                pssp.tile([128, 1024], F32, tag="sp", name=f"pq01_{i}")
                for i in range(2)
            ] + [
                psy.tile([128, 1024], F32, tag="y", name=f"pq01_{2 + i}")
                for i in range(2)
            ]
            for k in range(8):
                for mi in range(2):
                    for half in range(2):
                        for j in range(2):
                            c = half * 1024 + j * 512
                            nc.tensor.matmul(
                                pq01[2 * mi + half][:, j * 512 : (j + 1) * 512],
                                wqms[mi][:, k, :],
                                xT[:, k, c : c + 512],
                                start=(k == 0),
                                stop=(k == 7),
                            )
            for mi in range(2):
                for half in range(2):
                    nc.vector.tensor_scalar_add(
                        qkvTp[0][:, mi, half * 1024 : half * 1024 + 1024],
                        pq01[2 * mi + half][:],
                        biasT[:, mi : mi + 1],
                    )
            del wqms[0], wqms[1]
            for half in range(2):
                emit_qkv_chunk(0, 2, half, wqms[2])
            del wqms[2]
            vaug0 = vaug_pool.tile([128, 2, 16 * 65], BF16, tag="vaug")
            nc.gpsimd.memset(vaug0[:], 1.0)
            emit_vtrans(0, vaug0)
            vaugs = {0: vaug0}

            # filler-work generator: qkv chunks + vtrans for p+1 emitted
            # lazily inside B(p)'s kb loop
            def make_filler(pnext):
                if pnext > 3:
                    def _none():
                        return
                    return _none
                state = {"stage": 0, "r": 0, "half": 0, "vkb": 0}

                def step():
                    st = state["stage"]
                    if st == 0:
                        # DMA all three weight slices up front
                        for r in range(3):
                            m = 3 * pnext + r
                            wqm = wq_pool.tile(
                                [128, 8, 128], BF16, tag="wqm", name=f"wqm{m}"
                            )
                            nc.sync.dma_start(wqm[:], wqkv[m])
                            wqms[m] = wqm
                        state["stage"] = 1
                    elif st == 1:
                        r, half = state["r"], state["half"]
                        emit_qkv_chunk(pnext, r, half, wqms[3 * pnext + r])
                        if half == 1:
                            del wqms[3 * pnext + r]
                            state["r"], state["half"] = r + 1, 0
                            if r + 1 == 3:
                                state["stage"] = 2
                        else:
                            state["half"] = 1
                    elif st == 2:
                        vaug = vaug_pool.tile([128, 2, 16 * 65], BF16, tag="vaug")
                        nc.gpsimd.memset(vaug[:], 1.0)
                        vaugs[pnext] = vaug
                        state["stage"] = 3
                    elif st == 3:
                        # 2 kb-blocks of the v-transpose per step (8 sub-steps)
                        kb0 = state["vkb"]
                        emit_vtrans(pnext, vaugs[pnext], range(kb0, kb0 + 2))
                        state["vkb"] = kb0 + 2
                        if state["vkb"] == 16:
                            state["stage"] = 4
                    # stage 4: exhausted

                return step

            # ---------------- attention per head-pair p ----------------
            for p in range(4):
                filler = make_filler(p + 1)
                vaug = vaugs[p]
                nsteps = 0
                for qc in range(2):
                    qT = [qkvTp[p][64 * s : 64 * s + 64, 0, :] for s in range(2)]
                    kT = [qkvTp[p][64 * s : 64 * s + 64, 1, :] for s in range(2)]
                    kmax = (qc + 1) * 8
                    klast = [
                        min(kmax - 1, (qc * 2 + ci + 1) * 4 - 1) for ci in range(2)
                    ]
                    ymm = [
                        psy.tile([128, 1024], F32, tag="y", name=f"y{p}_{qc}_{s}")
                        for s in range(2)
                    ]
                    pend = []  # deferred PV work: (kb, q_lo, w, Pt pair)

                    def emit_pv(job):
                        kb_, q_lo_, w_, Pt_ = job
                        for s in range(2):
                            for j in range(w_ // 512):
                                col = q_lo_ - qc * 1024 + j * 512
                                ci = col // 512
                                nc.tensor.matmul(
                                    ymm[s][0:65, col : col + 512],
                                    vaug[:, s, kb_ * 65 : kb_ * 65 + 65],
                                    Pt_[s][:, j * 512 : (j + 1) * 512],
                                    start=(kb_ == 0),
                                    stop=(kb_ == klast[ci]),
                                )

                    for kb in range(kmax):
                        diag = kb >= qc * 8
                        q_lo = qc * 1024 if not diag else (kb * 128 // 512) * 512
                        w = (qc + 1) * 1024 - q_lo
                        dd = kb * 128 - q_lo if diag else 0  # in {0,128,256,384}
                        # --- scores, s0/s1 paired for row-group overlap
                        sp = []
                        for s in range(2):
                            spt = pssp.tile(
                                [128, 1024], F32, tag="sp", name=f"sp{p}_{qc}_{kb}_{s}"
                            )
                            sp.append(spt)
                            for j in range(w // 512):
                                c0 = j * 512 + (dd if j == 0 else 0)
                                c1 = (j + 1) * 512
                                nc.tensor.matmul(
                                    spt[:, c0:c1],
                                    kT[s][:, kb * 128 : (kb + 1) * 128],
                                    qT[s][:, q_lo + c0 : q_lo + c1],
                                    start=True,
                                    stop=True,
                                    tile_position=(64 * s, 0),
                                )
                        # --- exp (trimmed to causal range) + mask
                        Pt = []
                        for s in range(2):
                            Pts = P_pool.tile(
                                [128, 1024], BF16, tag="P", name=f"P{p}_{qc}_{kb}_{s}"
                            )
                            Pt.append(Pts)
                            nc.scalar.activation(
                                Pts[:, dd:w], sp[s][:, dd:w], AF.Exp, scale=0.125
                            )
                            if diag:
                                if dd > 0:
                                    nc.vector.memset(Pts[:, 0:dd], 0.0)
                                nc.vector.tensor_mul(
                                    Pts[:, dd : dd + 128],
                                    Pts[:, dd : dd + 128],
                                    tri[:],
                                )
                        # --- PV accumulate, two kbs behind the scores stream
                        # (full width; masked zeros above diag)
                        pend.append((kb, q_lo, w, Pt))
                        if len(pend) > 3:
                            emit_pv(pend.pop(0))
                        # spread the qkv filler work evenly across the kb loop
                        nsteps += 1
                        if nsteps % 3 != 0:
                            filler()
                    for job in pend:
                        emit_pv(job)
                    # --- normalize: y = ymm[0:64] / denom
                    for s in range(2):
                        den = smallB.tile([1, 1024], F32, tag="den")
                        nc.vector.tensor_copy(den[0:1, :], ymm[s][64:65, :])
                        rec = smallB.tile([1, 1024], F32, tag="rec")
                        nc.vector.reciprocal_approx_fast(rec[0:1, :], den[0:1, :])
                        bcs = smallB.tile([64, 1024], F32, tag="bcs")
                        nc.gpsimd.partition_broadcast(bcs[:], rec[0:1, :])
                        nc.vector.tensor_tensor(
                            out=yTp[p][
                                64 * s : 64 * s + 64, qc * 1024 : (qc + 1) * 1024
                            ],
                            in0=ymm[s][0:64, :],
                            in1=bcs[:],
                            op=MUL,
                        )
                # drain any remaining filler work before moving on
                for _ in range(24):
                    filler()

        # ---------------- output projection ----------------
        with (
            tc.tile_pool(name="ob", bufs=2) as ob_pool,
            tc.tile_pool(name="psC", bufs=4, space="PSUM") as psC,
        ):
            for m in range(8):
                pn = [
                    psC.tile([128, 512], F32, tag="pc", name=f"pc{m}_{n}")
                    for n in range(4)
                ]
                for k in range(4):
                    for n in range(4):
                        nc.tensor.matmul(
                            pn[n][:],
                            wps[:, k, m * 128 : (m + 1) * 128],
                            yTp[k][:, n * 512 : (n + 1) * 512],
                            start=(k == 0),
                            stop=(k == 3),
                        )
                ob = ob_pool.tile([128, T], BF16, tag="ob")
                for n in range(4):
                    nc.vector.tensor_copy(ob[:, n * 512 : (n + 1) * 512], pn[n][:])
                nc.sync.dma_start(outT[m * 128 : (m + 1) * 128, :], ob[:])

    nc.compile()
    return nc


def _get_nc():
    if "nc" not in _CACHE:
        _CACHE["nc"] = _build()
    return _CACHE["nc"]


def _prep_core_inputs(x, w_attn, b_attn, w_proj, b, g):
    cols = []
    for p in range(4):
        off = 512 * g + 128 * p
        cols += [
            w_attn[:, off : off + 128],
            w_attn[:, E + off : E + off + 128],
            w_attn[:, 2 * E + off : 2 * E + off + 128],
        ]
    wq = np.concatenate(cols, axis=1)  # [1024, 1536]
    # -> [12, 128, 8, 128]: m-major so each per-m DMA slice is contiguous
    wq = np.ascontiguousarray(
        wq.reshape(8, 128, 12, 128).transpose(2, 1, 0, 3), dtype=np.float32
    )
    bcols = []
    for p in range(4):
        off = 512 * g + 128 * p
        bcols += [
            b_attn[off : off + 128],
            b_attn[E + off : E + off + 128],
            b_attn[2 * E + off : 2 * E + off + 128],
        ]
    bq = np.stack(bcols, axis=1).astype(np.float32)  # [128, 12]
    wpr = np.concatenate(
        [w_proj[512 * g + 128 * p : 512 * g + 128 * p + 128, :] for p in range(4)],
        axis=0,
    )  # [512, 1024]
    wpr = np.ascontiguousarray(
        wpr.reshape(4, 128, 1024).transpose(1, 0, 2), dtype=np.float32
    )
    return {
        "xinT": np.ascontiguousarray(x[b].T).astype(ml_dtypes.bfloat16),
        "wqkv": wq.astype(ml_dtypes.bfloat16),
        "bqkv": np.ascontiguousarray(bq),
        "wp": wpr.astype(ml_dtypes.bfloat16),
    }


def kernel(x, w_attn, b_attn, w_proj, b_proj, _trace=False):
    from concourse.bass_utils import run_bass_kernel_spmd

    x = np.asarray(x, dtype=np.float32)
    w_attn = np.asarray(w_attn, dtype=np.float32)
    b_attn = np.asarray(b_attn, dtype=np.float32)
    w_proj = np.asarray(w_proj, dtype=np.float32)
    b_proj = np.asarray(b_proj, dtype=np.float32)

    nc = _get_nc()
    in_maps = [
        _prep_core_inputs(x, w_attn, b_attn, w_proj, core // 2, core % 2)
        for core in range(8)
    ]
    res = run_bass_kernel_spmd(
        nc, in_maps, core_ids=list(range(8)), trace=_trace
    )
    _CACHE["last_results"] = res
    out = np.empty((B, T, E), dtype=np.float32)
    for b in range(B):
        acc = res.results[2 * b]["outT"].astype(np.float32) + res.results[
            2 * b + 1
        ]["outT"].astype(np.float32)
        out[b] = acc.T + b_proj[None, :]
    return out
